# revision 2
# baseline (speedup 1.0000x reference)
"""Trainium2 Bass kernel for nn_Attention_73538430042164: baseline-grade f32r numerics + chunked pipeline.

Per core (batch -> core; m=n=1024, d=256), all data at f32r (fp22/11-bit
mantissa on HW), state path exact fp32 through PSUM + DVE custom ops:

  per (j,h) 512-col chunk:
    ps2'[j,h] = OP_QPS(ps2[j,h], pt32)          # clip01(s) - pt, fp32, DVE->PSUM
    ps2'[j,h] -= nc1t^T t1r                     # f32r matmuls (2 dh), skip_group_check
    rhs[j,h]  = OP_RHSPS(ps2[j,h], pt32)        # 2clip01(s)-s-pt, f32r out
  t1' = sum_j vsw^T rhs                         # f32r matmuls into t1 psum
  t1r = ACT copy of t1 psum (f32r)
  last iter: s32 = ACT(ps2) -> coeff16 = s32 > 0.5; out = rownorm(coeff)^T [Vs|1]
"""

import numpy as np

M, N, D = 1024, 1024, 256
B = 8
LAM = 0.1
N_ITERS = 50
T_BIAS = float(np.float32(LAM) / np.float32(M))

_CACHE = {}


def _register_dve_ops():
    import concourse.dve_ops as dve_ops

    if "ADMM_RHSP3_ANT" in dve_ops._SUB_OPCODE_FOR_NAME:
        return (
            [op for op in dve_ops.OPS if op.name == "ADMM_RHSP3_ANT"][0],
            [op for op in dve_ops.OPS if op.name == "ADMM_QP3_ANT"][0],
        )

    from concourse.dve_spec import Spec, Src0, Src1, Zero, One, maxx, minn, lower, _has_src1
    from concourse.dve_uop import DveOpSpec

    def reg(name, spec):
        opcode = dve_ops._CUSTOM_DVE_ROW_BASE + len(dve_ops.OPS)
        assert opcode < 0x20
        shas = {}
        for ver in ("v3", "v4"):
            s = DveOpSpec(name=name, opcode=opcode, uops=lower(spec, ver=ver),
                          rd1_en=_has_src1(spec))
            shas[ver] = s.sha(ver)
        op = dve_ops.DveOp(name, spec, subdim=False, uops_sha=shas)
        dve_ops.OPS.append(op)
        dve_ops.CUSTOM_DVE_SPECS[name] = spec
        dve_ops._SUB_OPCODE_FOR_NAME[name] = opcode
        return op

    z = minn(maxx(Src0, Zero), One)
    op_rhs = reg("ADMM_RHSP3_ANT", Spec(
        body=z + z - Src0 - Src1,
        reference=lambda in0, in1, s0, s1, imm2:
            (2.0 * np.clip(in0, 0.0, 1.0) - in0 - in1).astype(np.float32),
    ))
    z2 = minn(maxx(Src0, Zero), One)
    op_q = reg("ADMM_QP3_ANT", Spec(
        body=z2 - Src1,
        reference=lambda in0, in1, s0, s1, imm2:
            (np.clip(in0, 0.0, 1.0) - in1).astype(np.float32),
    ))
    return op_rhs, op_q


def build_nc():
    if "nc" in _CACHE:
        return _CACHE["nc"]

    import concourse.bacc as bacc
    import concourse.mybir as mybir
    import concourse.tile as tile

    OP_RHSPS, OP_QPS = _register_dve_ops()

    f32 = mybir.dt.float32
    f32r = mybir.dt.float32r
    f16 = mybir.dt.float16
    i32 = mybir.dt.int32
    Alu = mybir.AluOpType
    Act = mybir.ActivationFunctionType

    nc = bacc.Bacc("TRN2", target_bir_lowering=False, debug=False)
    Qd = nc.dram_tensor("q_in", [N, D], f32, kind="ExternalInput").ap()
    Vd = nc.dram_tensor("v_in", [M, D], f32, kind="ExternalInput").ap()
    Od = nc.dram_tensor("o_out", [N, D], f32, kind="ExternalOutput").ap()

    JT, NT, DT, NH = M // 128, N // 128, D // 128, N // 512
    n_it = _CACHE.get("debug_k") or N_ITERS

    with tile.TileContext(nc) as tc:
        with (
            tc.tile_pool(name="const", bufs=1) as cpool,
            tc.tile_pool(name="state", bufs=1) as spool,
            tc.tile_pool(name="ps", bufs=4, space="PSUM") as pspool,
            tc.tile_pool(name="t1p", bufs=1, space="PSUM") as t1pool,
        ):
            def h512(ap, h):
                return ap[:, h * 512:(h + 1) * 512]

            vs = [cpool.tile([128, D], f32r, tag=f"vs{j}", name=f"vs{j}") for j in range(JT)]
            vsf = [cpool.tile([128, D], f32, tag=f"vsf{j}", name=f"vsf{j}") for j in range(JT)]
            vstf = [cpool.tile([128, M], f32, tag=f"vstf{h}", name=f"vstf{h}") for h in range(DT)]
            vst = [cpool.tile([128, M], f32r, tag=f"vst{h}", name=f"vst{h}") for h in range(DT)]
            qt = [cpool.tile([128, N], f32, tag=f"qt{h}", name=f"qt{h}") for h in range(DT)]
            nc1t = [cpool.tile([128, M], f32r, tag=f"nc1t{h}", name=f"nc1t{h}") for h in range(DT)]
            vsaug16 = [cpool.tile([128, D + 2], f16, tag=f"va{j}", name=f"va{j}") for j in range(JT)]
            ident = cpool.tile([128, 128], f32r, tag="ident", name="ident")
            identf = cpool.tile([128, 128], f32, tag="identf", name="identf")
            i256 = [cpool.tile([128, D], f32, tag=f"i256_{h}", name=f"i256_{h}") for h in range(DT)]
            iot = cpool.tile([128, 128], i32, tag="iot", name="iot")
            rinv = [cpool.tile([128, 1], f32, tag=f"rinv{t}", name=f"rinv{t}") for t in range(NT)]
            rsum = [cpool.tile([128, 1], f32, tag=f"rsum{t}", name=f"rsum{t}") for t in range(NT)]

            pt32 = spool.tile([128, JT * N], f32, tag="pt32", name="pt32")
            stage = spool.tile([128, JT * 512], f32, tag="stage", name="stage")
            rhsr = spool.tile([128, JT * N], f32r, tag="rhsr", name="rhsr")
            t1r = spool.tile([128, DT * N], f32r, tag="t1r", name="t1r")
            s32 = spool.tile([128, JT * N], f32, tag="s32", name="s32")
            coeff16 = stage[:].bitcast(f16)            # overlay: stage is dead post-setup
            outsb = [t1r[:, t * D:(t + 1) * D] for t in range(NT)]

            def jn(ap, j):
                return ap[:, j * N:(j + 1) * N]

            # ---------------- setup ----------------
            nc.gpsimd.iota(iot[:], pattern=[[1, 128]], base=127, channel_multiplier=-1)
            nc.vector.tensor_scalar(identf[:], iot[:], 127, None, Alu.is_equal)
            nc.vector.tensor_scalar_mul(ident[:], identf[:], 1.0)
            for h in range(DT):
                nc.vector.memset(i256[h][:], 0.0)
                nc.vector.tensor_copy(i256[h][:, h * 128:(h + 1) * 128], identf[:])

            for j in range(JT):
                sv = stage[:, j * 512:j * 512 + D]
                sq = stage[:, j * 512 + D:(j + 1) * 512]
                nc.sync.dma_start(sv, Vd[j * 128:(j + 1) * 128, :])
                nc.sync.dma_start(sq, Qd[j * 128:(j + 1) * 128, :])
                nc.vector.tensor_scalar_mul(vs[j][:], sv, 1.0 / M)
                nc.vector.tensor_scalar_mul(vsf[j][:], sv, 1.0 / M)
                nc.scalar.activation(vsaug16[j][:, :D], vsf[j][:], Act.Copy)
                nc.vector.tensor_scalar(vsaug16[j][:, D:D + 1], identf[:, :1], 0.0, 1.0,
                                        Alu.mult, Alu.add)
                nc.vector.tensor_scalar(vsaug16[j][:, D + 1:D + 2], identf[:, :1], 0.0, 0.0,
                                        Alu.mult, Alu.add)

            for j in range(JT):
                for h in range(DT):
                    tp = pspool.tile([128, 128], f32r, tag="ps", name=f"tpv{j}_{h}")
                    nc.tensor.transpose(tp[:], vs[j][:, h * 128:(h + 1) * 128], ident[:])
                    nc.scalar.copy(vst[h][:, j * 128:(j + 1) * 128], tp[:])
                    tf = pspool.tile([128, 128], f32, tag="ps", name=f"tpf{j}_{h}")
                    nc.tensor.transpose(tf[:], vsf[j][:, h * 128:(h + 1) * 128], identf[:])
                    nc.scalar.copy(vstf[h][:, j * 128:(j + 1) * 128], tf[:])
                    tq = pspool.tile([128, 128], f32, tag="ps", name=f"tpq{j}_{h}")
                    nc.tensor.transpose(tq[:], stage[:, j * 512 + D + h * 128:j * 512 + D + (h + 1) * 128], identf[:])
                    nc.scalar.copy(qt[h][:, j * 128:(j + 1) * 128], tq[:])

            A = [cpool.tile([128, D], f32r, tag=f"A{h}", name=f"A{h}") for h in range(DT)]
            for dh in range(DT):
                tps = pspool.tile([128, D], f32, tag="ps", name=f"Tps{dh}")
                for j in range(JT):
                    nc.tensor.matmul(
                        tps[:], vs[j][:, dh * 128:(dh + 1) * 128],
                        vs[j][:], start=(j == 0), stop=(j == JT - 1))
                nc.vector.tensor_scalar_mul(A[dh][:], tps[:], 2.0)

            Xc = [cpool.tile([128, D], f32r, tag=f"X{h}", name=f"X{h}") for h in range(DT)]
            Xn = [cpool.tile([128, D], f32r, tag=f"Xn{h}", name=f"Xn{h}") for h in range(DT)]
            for h in range(DT):
                nc.vector.tensor_sub(Xc[h][:], i256[h][:], A[h][:])
            for it in range(3):
                for dh in range(DT):
                    yp = pspool.tile([128, D], f32, tag="ps", name=f"neu{it}_{dh}")
                    for kh in range(DT):
                        nc.tensor.matmul(
                            yp[:], A[kh][:, dh * 128:(dh + 1) * 128],
                            Xc[kh][:], start=(kh == 0), stop=(kh == DT - 1))
                    nc.vector.tensor_sub(Xn[dh][:], i256[dh][:], yp[:])
                for h in range(DT):
                    nc.vector.tensor_copy(Xc[h][:], Xn[h][:])
            negs2 = [cpool.tile([128, D], f32r, tag=f"ns2{h}", name=f"ns2{h}") for h in range(DT)]
            for h in range(DT):
                nc.vector.tensor_scalar_mul(negs2[h][:], Xc[h][:], -2.0)

            # nc1t = -S2inv Vs^T (f32r weights for mm2)
            for dh in range(DT):
                for h in range(NH):
                    big = pspool.tile([128, 512], f32, tag="ps", name=f"c1p{dh}_{h}")
                    for kh in range(DT):
                        nc.tensor.matmul(
                            big[:], negs2[kh][:, dh * 128:(dh + 1) * 128],
                            h512(vst[kh], h), start=(kh == 0), stop=(kh == DT - 1))
                    nc.scalar.copy(nc1t[dh][:, h * 512:(h + 1) * 512], big[:])

            # pt32 = -2 Vs Q^T + T
            for j in range(JT):
                for h in range(NH):
                    pp = pspool.tile([128, 512], f32, tag="ps", name=f"ptp{j}_{h}")
                    for kh in range(DT):
                        nc.tensor.matmul(
                            pp[:], vstf[kh][:, j * 128:(j + 1) * 128],
                            h512(qt[kh], h), start=(kh == 0), stop=(kh == DT - 1))
                    nc.vector.tensor_scalar(h512(jn(pt32, j), h), pp[:], -2.0,
                                            T_BIAS, Alu.mult, Alu.add)

            # rhs_0 = -pt (f32r, DVE write rounds); t1_0 = Vs^T rhs_0
            for j in range(JT):
                nc.vector.tensor_scalar_mul(jn(rhsr, j), jn(pt32, j), -1.0)
            t1ps = [t1pool.tile([128, N], f32, tag=f"t1_{dh}", name=f"t1_{dh}") for dh in range(DT)]
            for dh in range(DT):
                for h in range(NH):
                    for i in range(JT):
                        nc.tensor.matmul(
                            h512(t1ps[dh], h),
                            vs[i][:, dh * 128:(dh + 1) * 128],
                            jn(rhsr, i)[:, h * 512:(h + 1) * 512],
                            start=(i == 0), stop=(i == JT - 1))

            # ---------------- ADMM iterations ----------------
            def emit_mm1(jp):
                for h in range(NH):
                    for dh in range(DT):
                        for par in range(2):
                            j = 2 * jp + par
                            nc.tensor.matmul(
                                t1ps[dh][:, h * 512:(h + 1) * 512],
                                vs[j][:, dh * 128:(dh + 1) * 128],
                                h512(jn(rhsr, j), h),
                                start=(jp == 0 and par == 0),
                                stop=(jp == JT // 2 - 1 and par == 1))

            for k in range(n_it):
                last = (k == n_it - 1)
                for h in range(NH):
                    for dh in range(DT):
                        nc.scalar.copy(t1r[:, dh * N + h * 512:dh * N + (h + 1) * 512],
                                       t1ps[dh][:, h * 512:(h + 1) * 512])
                pend_mm1 = []
                for j in range(JT):
                    for h in range(NH):
                        ps2 = pspool.tile([128, 512], f32, tag="ps", name=f"ps2_{k}_{j}_{h}")
                        sj = h512(jn(s32, j), h)
                        pj = h512(jn(pt32, j), h)
                        if k == 0:
                            nc.vector.tensor_scalar_mul(ps2[:], pj, -1.0)
                        else:
                            nc.vector._custom_dve(OP_QPS, out=ps2[:], in0=sj, in1=pj)
                            if not last:
                                nc.vector._custom_dve(OP_RHSPS, out=h512(jn(rhsr, j), h),
                                                      in0=sj, in1=pj)
                        for dh in range(DT):
                            nc.tensor.matmul(
                                ps2[:], nc1t[dh][:, j * 128:(j + 1) * 128],
                                t1r[:, dh * N + h * 512:dh * N + (h + 1) * 512],
                                start=False, stop=(dh == DT - 1), skip_group_check=True)
                        # s_{k+1} back to SBUF (overwrites old s after its readers)
                        nc.scalar.copy(sj, ps2[:])
                    if last:
                        nc.vector.tensor_scalar(jn(coeff16, j), jn(s32, j), 0.5, None, Alu.is_gt)
                    else:
                        if j % 2 == 1:
                            pend_mm1.append(j // 2)
                        while pend_mm1 and (len(pend_mm1) >= 2 or j == JT - 1):
                            emit_mm1(pend_mm1.pop(0))

            # ---------------- output ----------------
            if _CACHE.get("debug_k") is not None:
                for jj in range(2):
                    nc.sync.dma_start(Od[jj * 512:(jj + 1) * 512, :],
                                      s32[:, jj * N:(jj + 1) * N])
            else:
                for t in range(NT):
                    o2 = pspool.tile([128, D + 2], f32, tag="ps", name=f"o2_{t}")
                    for j in range(JT):
                        nc.tensor.matmul(
                            o2[:], jn(coeff16, j)[:, t * 128:(t + 1) * 128],
                            vsaug16[j][:], start=(j == 0), stop=(j == JT - 1))
                    nc.vector.tensor_scalar_add(rsum[t][:], o2[:, D:D + 1], 1e-10)
                    nc.vector.reciprocal(rinv[t][:], rsum[t][:])
                    nc.scalar.activation(outsb[t], o2[:, :D], Act.Copy, scale=rinv[t][:])
                    nc.sync.dma_start(Od[t * 128:(t + 1) * 128, :], outsb[t].bitcast(f32))

    nc.compile()
    _CACHE["nc"] = nc
    return nc


def run(Q, V, trace=False, trace_kwargs=None):
    from concourse import bass_utils

    nc = build_nc()
    Q = np.ascontiguousarray(np.asarray(Q, dtype=np.float32))
    V = np.ascontiguousarray(np.asarray(V, dtype=np.float32))
    assert Q.shape == (B, N, D) and V.shape == (B, M, D)
    in_maps = [{"q_in": Q[i], "v_in": V[i]} for i in range(B)]
    res = bass_utils.run_bass_kernel_spmd(
        nc, in_maps, core_ids=list(range(B)), trace=trace,
        trace_kwargs=trace_kwargs or {})
    out = np.stack([r["o_out"] for r in res.results]).astype(np.float32)
    return out, res


def kernel(Q, V):
    out, _ = run(Q, V)
    return out


# revision 8
# speedup vs baseline: 1.4592x; 1.4592x over previous
"""Trainium2 Bass kernel for nn_Attention_73538430042164: baseline-grade f32r numerics + chunked pipeline.

Per core (batch -> core; m=n=1024, d=256), all data at f32r (fp22/11-bit
mantissa on HW), state path exact fp32 through PSUM + DVE custom ops:

  per (j,h) 512-col chunk:
    ps2'[j,h] = OP_QPS(ps2[j,h], pt32)          # clip01(s) - pt, fp32, DVE->PSUM
    ps2'[j,h] -= nc1t^T t1r                     # f32r matmuls (2 dh), skip_group_check
    rhs[j,h]  = OP_RHSPS(ps2[j,h], pt32)        # 2clip01(s)-s-pt, f32r out
  t1' = sum_j vsw^T rhs                         # f32r matmuls into t1 psum
  t1r = ACT copy of t1 psum (f32r)
  last iter: s32 = ACT(ps2) -> coeff16 = s32 > 0.5; out = rownorm(coeff)^T [Vs|1]
"""

import numpy as np

M, N, D = 1024, 1024, 256
B = 8
LAM = 0.1
N_ITERS = 50
T_BIAS = float(np.float32(LAM) / np.float32(M))

_CACHE = {}


def _register_dve_ops():
    import concourse.dve_ops as dve_ops

    if "ADMM_RHSP3_ANT" in dve_ops._SUB_OPCODE_FOR_NAME:
        return (
            [op for op in dve_ops.OPS if op.name == "ADMM_RHSP3_ANT"][0],
            [op for op in dve_ops.OPS if op.name == "ADMM_QP3_ANT"][0],
        )

    from concourse.dve_spec import Spec, Src0, Src1, Zero, One, maxx, minn, lower, _has_src1
    from concourse.dve_uop import DveOpSpec

    def reg(name, spec):
        opcode = dve_ops._CUSTOM_DVE_ROW_BASE + len(dve_ops.OPS)
        assert opcode < 0x20
        shas = {}
        for ver in ("v3", "v4"):
            s = DveOpSpec(name=name, opcode=opcode, uops=lower(spec, ver=ver),
                          rd1_en=_has_src1(spec))
            shas[ver] = s.sha(ver)
        op = dve_ops.DveOp(name, spec, subdim=False, uops_sha=shas)
        dve_ops.OPS.append(op)
        dve_ops.CUSTOM_DVE_SPECS[name] = spec
        dve_ops._SUB_OPCODE_FOR_NAME[name] = opcode
        return op

    z = minn(maxx(Src0, Zero), One)
    op_rhs = reg("ADMM_RHSP3_ANT", Spec(
        body=z + z - Src0 - Src1,
        reference=lambda in0, in1, s0, s1, imm2:
            (2.0 * np.clip(in0, 0.0, 1.0) - in0 - in1).astype(np.float32),
    ))
    z2 = minn(maxx(Src0, Zero), One)
    op_q = reg("ADMM_QP3_ANT", Spec(
        body=z2 - Src1,
        reference=lambda in0, in1, s0, s1, imm2:
            (np.clip(in0, 0.0, 1.0) - in1).astype(np.float32),
    ))
    return op_rhs, op_q


def build_nc():
    if "nc" in _CACHE:
        return _CACHE["nc"]

    import concourse.bacc as bacc
    import concourse.mybir as mybir
    import concourse.tile as tile

    OP_RHSPS, OP_QPS = _register_dve_ops()

    f32 = mybir.dt.float32
    f32r = mybir.dt.float32r
    f16 = mybir.dt.float16
    i32 = mybir.dt.int32
    Alu = mybir.AluOpType
    Act = mybir.ActivationFunctionType

    nc = bacc.Bacc("TRN2", target_bir_lowering=False, debug=False)
    Qd = nc.dram_tensor("q_in", [N, D], f32, kind="ExternalInput").ap()
    Vd = nc.dram_tensor("v_in", [M, D], f32, kind="ExternalInput").ap()
    Od = nc.dram_tensor("o_out", [N, D], f32, kind="ExternalOutput").ap()

    JT, NT, DT, NH = M // 128, N // 128, D // 128, N // 512
    n_it = _CACHE.get("debug_k") or N_ITERS

    with tile.TileContext(nc) as tc:
        with (
            tc.tile_pool(name="const", bufs=1) as cpool,
            tc.tile_pool(name="state", bufs=1) as spool,
            tc.tile_pool(name="ps", bufs=2, space="PSUM") as pspool,
            tc.tile_pool(name="t1p", bufs=1, space="PSUM") as t1pool,
        ):
            def h512(ap, h):
                return ap[:, h * 512:(h + 1) * 512]

            vs = [cpool.tile([128, D], f32r, tag=f"vs{j}", name=f"vs{j}") for j in range(JT)]
            vsf = [cpool.tile([128, D], f32, tag=f"vsf{j}", name=f"vsf{j}") for j in range(JT)]
            vstf = [cpool.tile([128, M], f32, tag=f"vstf{h}", name=f"vstf{h}") for h in range(DT)]
            vst = [cpool.tile([128, M], f32r, tag=f"vst{h}", name=f"vst{h}") for h in range(DT)]
            qt = [cpool.tile([128, N], f32, tag=f"qt{h}", name=f"qt{h}") for h in range(DT)]
            nc1t = [cpool.tile([128, M], f32r, tag=f"nc1t{h}", name=f"nc1t{h}") for h in range(DT)]
            vsaug16 = [cpool.tile([128, D + 2], f16, tag=f"va{j}", name=f"va{j}") for j in range(JT)]
            ident = cpool.tile([128, 128], f32r, tag="ident", name="ident")
            identf = cpool.tile([128, 128], f32, tag="identf", name="identf")
            i256 = [cpool.tile([128, D], f32, tag=f"i256_{h}", name=f"i256_{h}") for h in range(DT)]
            iot = cpool.tile([128, 128], i32, tag="iot", name="iot")
            rinv = [cpool.tile([128, 1], f32, tag=f"rinv{t}", name=f"rinv{t}") for t in range(NT)]
            rsum = [cpool.tile([128, 1], f32, tag=f"rsum{t}", name=f"rsum{t}") for t in range(NT)]

            pt32 = spool.tile([128, JT * N], f32, tag="pt32", name="pt32")
            stage = spool.tile([128, JT * 512], f32, tag="stage", name="stage")
            rhsr = spool.tile([128, JT * N], f32r, tag="rhsr", name="rhsr")
            t1r = spool.tile([128, DT * N], f32r, tag="t1r", name="t1r")
            s32 = spool.tile([128, JT * N], f32, tag="s32", name="s32")
            qtmp = spool.tile([128, 2 * N], f32, tag="qtmp", name="qtmp")
            coeff16 = stage[:].bitcast(f16)            # overlay: stage is dead post-setup
            outsb = [t1r[:, t * D:(t + 1) * D] for t in range(NT)]

            def jn(ap, j):
                return ap[:, j * N:(j + 1) * N]

            # ---------------- setup ----------------
            nc.gpsimd.iota(iot[:], pattern=[[1, 128]], base=127, channel_multiplier=-1)
            nc.vector.tensor_scalar(identf[:], iot[:], 127, None, Alu.is_equal)
            nc.vector.tensor_scalar_mul(ident[:], identf[:], 1.0)
            for h in range(DT):
                nc.vector.memset(i256[h][:], 0.0)
                nc.vector.tensor_copy(i256[h][:, h * 128:(h + 1) * 128], identf[:])

            for j in range(JT):
                sv = stage[:, j * 512:j * 512 + D]
                sq = stage[:, j * 512 + D:(j + 1) * 512]
                nc.sync.dma_start(sv, Vd[j * 128:(j + 1) * 128, :])
                nc.sync.dma_start(sq, Qd[j * 128:(j + 1) * 128, :])
                nc.vector.tensor_scalar_mul(vs[j][:], sv, 1.0 / M)
                nc.vector.tensor_scalar_mul(vsf[j][:], sv, 1.0 / M)
                nc.scalar.activation(vsaug16[j][:, :D], vsf[j][:], Act.Copy)
                nc.vector.tensor_scalar(vsaug16[j][:, D:D + 1], identf[:, :1], 0.0, 1.0,
                                        Alu.mult, Alu.add)
                nc.vector.tensor_scalar(vsaug16[j][:, D + 1:D + 2], identf[:, :1], 0.0, 0.0,
                                        Alu.mult, Alu.add)

            for j in range(JT):
                for h in range(DT):
                    tp = pspool.tile([128, 128], f32r, tag="ps2", name=f"tpv{j}_{h}")
                    nc.tensor.transpose(tp[:], vs[j][:, h * 128:(h + 1) * 128], ident[:])
                    nc.scalar.copy(vst[h][:, j * 128:(j + 1) * 128], tp[:])
                    tf = pspool.tile([128, 128], f32, tag="ps2", name=f"tpf{j}_{h}")
                    nc.tensor.transpose(tf[:], vsf[j][:, h * 128:(h + 1) * 128], identf[:])
                    nc.scalar.copy(vstf[h][:, j * 128:(j + 1) * 128], tf[:])
                    tq = pspool.tile([128, 128], f32, tag="ps2", name=f"tpq{j}_{h}")
                    nc.tensor.transpose(tq[:], stage[:, j * 512 + D + h * 128:j * 512 + D + (h + 1) * 128], identf[:])
                    nc.scalar.copy(qt[h][:, j * 128:(j + 1) * 128], tq[:])

            A = [cpool.tile([128, D], f32r, tag=f"A{h}", name=f"A{h}") for h in range(DT)]
            for dh in range(DT):
                tps = pspool.tile([128, D], f32, tag="ps2", name=f"Tps{dh}")
                for j in range(JT):
                    nc.tensor.matmul(
                        tps[:], vs[j][:, dh * 128:(dh + 1) * 128],
                        vs[j][:], start=(j == 0), stop=(j == JT - 1))
                nc.vector.tensor_scalar_mul(A[dh][:], tps[:], 2.0)

            Xc = [cpool.tile([128, D], f32r, tag=f"X{h}", name=f"X{h}") for h in range(DT)]
            Xn = [cpool.tile([128, D], f32r, tag=f"Xn{h}", name=f"Xn{h}") for h in range(DT)]
            for h in range(DT):
                nc.vector.tensor_sub(Xc[h][:], i256[h][:], A[h][:])
            for it in range(3):
                for dh in range(DT):
                    yp = pspool.tile([128, D], f32, tag="ps2", name=f"neu{it}_{dh}")
                    for kh in range(DT):
                        nc.tensor.matmul(
                            yp[:], A[kh][:, dh * 128:(dh + 1) * 128],
                            Xc[kh][:], start=(kh == 0), stop=(kh == DT - 1))
                    nc.vector.tensor_sub(Xn[dh][:], i256[dh][:], yp[:])
                for h in range(DT):
                    nc.vector.tensor_copy(Xc[h][:], Xn[h][:])
            negs2 = [cpool.tile([128, D], f32r, tag=f"ns2{h}", name=f"ns2{h}") for h in range(DT)]
            for h in range(DT):
                nc.vector.tensor_scalar_mul(negs2[h][:], Xc[h][:], -2.0)

            # nc1t = -S2inv Vs^T (f32r weights for mm2)
            for dh in range(DT):
                for h in range(NH):
                    big = pspool.tile([128, 512], f32, tag="ps2", name=f"c1p{dh}_{h}")
                    for kh in range(DT):
                        nc.tensor.matmul(
                            big[:], negs2[kh][:, dh * 128:(dh + 1) * 128],
                            h512(vst[kh], h), start=(kh == 0), stop=(kh == DT - 1))
                    nc.scalar.copy(nc1t[dh][:, h * 512:(h + 1) * 512], big[:])

            # pt32 = -2 Vs Q^T + T
            for j in range(JT):
                for h in range(NH):
                    pp = pspool.tile([128, 512], f32, tag="ps2", name=f"ptp{j}_{h}")
                    for kh in range(DT):
                        nc.tensor.matmul(
                            pp[:], vstf[kh][:, j * 128:(j + 1) * 128],
                            h512(qt[kh], h), start=(kh == 0), stop=(kh == DT - 1))
                    nc.vector.tensor_scalar(h512(jn(pt32, j), h), pp[:], -2.0,
                                            T_BIAS, Alu.mult, Alu.add)

            # rhs_0 = -pt (f32r, DVE write rounds); t1_0 = Vs^T rhs_0
            for j in range(JT):
                nc.vector.tensor_scalar_mul(jn(rhsr, j), jn(pt32, j), -1.0)
            t1ps = [t1pool.tile([128, N], f32, tag=f"t1_{dh}", name=f"t1_{dh}") for dh in range(DT)]
            for dh in range(DT):
                for h in range(NH):
                    for i in range(JT):
                        nc.tensor.matmul(
                            h512(t1ps[dh], h),
                            vs[i][:, dh * 128:(dh + 1) * 128],
                            jn(rhsr, i)[:, h * 512:(h + 1) * 512],
                            start=(i == 0), stop=(i == JT - 1))

            # ---------------- ADMM iterations ----------------
            def emit_mm1(jp):
                for h in range(NH):
                    for dh in range(DT):
                        for par in range(2):
                            j = 2 * jp + par
                            nc.tensor.matmul(
                                t1ps[dh][:, h * 512:(h + 1) * 512],
                                vs[j][:, dh * 128:(dh + 1) * 128],
                                h512(jn(rhsr, j), h),
                                start=(jp == 0 and par == 0),
                                stop=(jp == JT // 2 - 1 and par == 1))

            POOL_J = ()   # Pool path races (wild rel-err variance across identical runs) - keep off
            for k in range(n_it):
                last = (k == n_it - 1)
                for h in range(NH):
                    for dh in range(DT):
                        nc.scalar.copy(t1r[:, dh * N + h * 512:dh * N + (h + 1) * 512],
                                       t1ps[dh][:, h * 512:(h + 1) * 512])
                if 0 < k:
                    # Pool precomputes q = clip(s) - pt for the offloaded j's
                    for i, j in enumerate(POOL_J):
                        sl = qtmp[:, i * N:(i + 1) * N]
                        nc.gpsimd.tensor_scalar(sl, jn(s32, j), 0.0, 1.0,
                                                Alu.max, Alu.min)
                        nc.gpsimd.tensor_tensor(sl, sl, jn(pt32, j), Alu.subtract)
                pend_mm1 = []
                for j in range(JT):
                    ps2 = pspool.tile([128, N], f32, tag="ps2", name=f"ps2_{k}_{j}")
                    sj = jn(s32, j)
                    pj = jn(pt32, j)
                    if k == 0:
                        nc.vector.tensor_scalar_mul(ps2[:], pj, -1.0)
                    else:
                        if j in POOL_J:
                            nc.scalar.copy(ps2[:], qtmp[:, POOL_J.index(j) * N:
                                                        (POOL_J.index(j) + 1) * N])
                        else:
                            nc.vector._custom_dve(OP_QPS, out=ps2[:], in0=sj, in1=pj)
                        if not last:
                            nc.vector._custom_dve(OP_RHSPS, out=jn(rhsr, j),
                                                  in0=sj, in1=pj)
                    for h in range(NH):
                        for dh in range(DT):
                            nc.tensor.matmul(
                                h512(ps2, h), nc1t[dh][:, j * 128:(j + 1) * 128],
                                t1r[:, dh * N + h * 512:dh * N + (h + 1) * 512],
                                start=False, stop=(dh == DT - 1), skip_group_check=True)
                    nc.scalar.copy(sj, ps2[:])
                    if last:
                        nc.vector.tensor_scalar(jn(coeff16, j), jn(s32, j), 0.5, None, Alu.is_gt)
                    else:
                        if j % 2 == 1:
                            pend_mm1.append(j // 2)
                        while pend_mm1 and (len(pend_mm1) >= 2 or j == JT - 1):
                            emit_mm1(pend_mm1.pop(0))

            # ---------------- output ----------------
            if _CACHE.get("debug_k") is not None:
                for jj in range(2):
                    nc.sync.dma_start(Od[jj * 512:(jj + 1) * 512, :],
                                      s32[:, jj * N:(jj + 1) * N])
            else:
                for t in range(NT):
                    o2 = pspool.tile([128, D + 2], f32, tag="ps2", name=f"o2_{t}")
                    for j in range(JT):
                        nc.tensor.matmul(
                            o2[:], jn(coeff16, j)[:, t * 128:(t + 1) * 128],
                            vsaug16[j][:], start=(j == 0), stop=(j == JT - 1))
                    nc.vector.tensor_scalar_add(rsum[t][:], o2[:, D:D + 1], 1e-10)
                    nc.vector.reciprocal(rinv[t][:], rsum[t][:])
                    nc.scalar.activation(outsb[t], o2[:, :D], Act.Copy, scale=rinv[t][:])
                    nc.sync.dma_start(Od[t * 128:(t + 1) * 128, :], outsb[t].bitcast(f32))

    nc.compile()
    _CACHE["nc"] = nc
    return nc


def run(Q, V, trace=False, trace_kwargs=None):
    from concourse import bass_utils

    nc = build_nc()
    Q = np.ascontiguousarray(np.asarray(Q, dtype=np.float32))
    V = np.ascontiguousarray(np.asarray(V, dtype=np.float32))
    assert Q.shape == (B, N, D) and V.shape == (B, M, D)
    in_maps = [{"q_in": Q[i], "v_in": V[i]} for i in range(B)]
    res = bass_utils.run_bass_kernel_spmd(
        nc, in_maps, core_ids=list(range(B)), trace=trace,
        trace_kwargs=trace_kwargs or {})
    out = np.stack([r["o_out"] for r in res.results]).astype(np.float32)
    return out, res


def kernel(Q, V):
    out, _ = run(Q, V)
    return out
